# revision 8
# baseline (speedup 1.0000x reference)
"""Trainium2 Bass kernel for masked-LSTM sentence classifier (nn_ABSA_Lstm).

Data-parallel over 8 NeuronCores, 128 sentences per core.

v3: gate column order [f | i | g | o] (host-permuted), one PSUM tile per
gate so the f-sigmoid starts after only its own 3 h-matmuls and the ACT
chain runs f -> i -> g -> o -> tanh(c) with the DVE cell update
(t2 = f*c, t1 = i*g, c = t1+t2, h = o*tanh(c)) interleaved.

Per core:
  - x path (pipelined LAG steps ahead): one indirect DMA per timestep
    gathers emb[sent[:,t]] from a bf16 embedding table (host-cast) into a
    ring of padded bf16 staging tiles; xbar DMA transposes (bursts, Sync
    queue reserved for them) produce xT with a constant ones-row so the
    gate bias rides along in the x matmul (Wx host-augmented with b,
    zero-padded to K=384).  Weight/const DMAs ride the tensor/scalar/
    vector engine queues so they don't delay the first transposes.
  - scan t = 0..T-1 (bf16 matmuls + elementwise, fp32 PSUM): x-parts for
    step t+1 prefill the f/i tiles (double-buffered) right after step t's
    h-matmuls as PE filler; g/o tiles are single-buffered and refill after
    their ACT readers.  h transposed on PE into ONE PSUM tile with ONE DVE
    copy so all 12 next-step h-matmuls become ready together (keeps the
    scheduler from k-major reordering them, which would delay sig_f).
    h captured into hout at t == len-1 via one fused per-partition
    mul-add, scheduling-pinned after the ht copy.
  - logit = hout @ Wout + bout (f32 matmul), one DMA out.

A post-pass splits multi-wait instructions (this walrus accepts one sync
wait per instruction): engine ops spill waits onto same-engine NoOps; DMA
queue ops route spilled waits through dedicated semaphores that are
restored to zero at the kernel tail (NEFF stays re-executable).
"""

import sys

for _p in ("/opt/trn_rl_repo", "/root/.axon_site/_ro/trn_rl_repo"):
    if _p not in sys.path:
        sys.path.append(_p)

import numpy as np
import ml_dtypes

from concourse import bass, mybir
import concourse.tile as tile
from concourse.bass import IndirectOffsetOnAxis
from concourse.bass_utils import run_bass_kernel_spmd
from concourse.masks import make_identity

B, T, V, D, H, C = 1024, 80, 50000, 300, 300, 3
G = 4 * H            # 1200 gate columns, order [f | i | g | o]
DA = D + 1           # 301: ones-row for bias
N_CORES = 8
BC = B // N_CORES    # 128 sentences per core
P = 128

F32 = mybir.dt.float32
BF16 = mybir.dt.bfloat16
I32 = mybir.dt.int32

KH = [(0, 128), (128, 256), (256, H)]        # h-side contraction tiles
# gate column ranges in the permuted weights
NF, NI, NG, NO = (0, H), (H, 2 * H), (2 * H, 3 * H), (3 * H, G)


def _sync_wait(sem_id, value):
    import bass_rust
    return bass_rust.SyncWait(
        sync_type="semaphore", id=sem_id, ant_name=f"splitsem_{sem_id}",
        wait_mode="sem-ge-imm", wait_value=value, wait_reg=None,
    )


def _sync_update(sem_id, mode, value):
    import bass_rust
    return bass_rust.SyncUpdate(
        sync_type="semaphore", id=sem_id, ant_name=f"splitsem_{sem_id}",
        update_mode=mode, update_value=value, update_reg=None,
    )


def _split_multi_waits(nc, spare_sem_ids):
    """walrus caps sync waits per instruction at 1 for every struct we hit.

    Engine instructions: spill excess waits onto single-wait NoOps placed
    just before, on the same engine (engine streams are in-order).

    DMA/queue instructions: a preceding engine NoOp may not order the DGE
    ring, so the spill NoOps perform ALL the original waits and the last one
    increments a dedicated semaphore; the DMA's single wait becomes that
    semaphore. Each such semaphore is decremented back to 0 at the kernel
    tail so repeated NEFF executions stay correct."""
    f = nc.m.functions[0]
    spare = list(spare_sem_ids)
    eng_sem = {}     # engine -> sem id (one per issuing engine, in-order stream)
    eng_count = {}   # engine -> number of increments so far
    for blk in f.blocks:
        out = []
        for ins in blk.instructions:
            si = ins.sync_info
            waits = list(si.on_wait) if si and si.on_wait else []
            if len(waits) <= 1:
                out.append(ins)
                continue
            tname = type(ins).__name__
            is_dma = ("DMA" in tname or "TensorLoad" in tname
                      or "TensorSave" in tname)
            if is_dma:
                eng = ins.engine
                if eng not in eng_sem:
                    eng_sem[eng] = spare.pop()
                    eng_count[eng] = 0
                sid = eng_sem[eng]
                eng_count[eng] += 1
                target = eng_count[eng]
                for j, w in enumerate(waits):
                    nop = mybir.InstNoOp(name=f"nop-dsplit-{nc.next_id()}")
                    nop.engine = eng
                    upd = [_sync_update(sid, "sem-inc", 1)] if j == len(waits) - 1 else []
                    nop.sync_info = mybir.SyncInfo(on_wait=[w], on_update=upd)
                    out.append(nop)
                si.on_wait = [_sync_wait(sid, target)]
            else:
                for w in waits[:-1]:
                    nop = mybir.InstNoOp(name=f"nop-split-{nc.next_id()}")
                    nop.engine = ins.engine
                    nop.sync_info = mybir.SyncInfo(on_wait=[w], on_update=[])
                    out.append(nop)
                si.on_wait = waits[-1:]
            out.append(ins)
        blk.instructions = out
    # tail: restore spilled-DMA semaphores to 0 for repeat executions
    if eng_sem:
        last_blk = f.blocks[-1]
        tail = list(last_blk.instructions)
        for eng, sid in eng_sem.items():
            nop = mybir.InstNoOp(name=f"nop-dclear-{nc.next_id()}")
            nop.engine = mybir.EngineType.SP
            nop.sync_info = mybir.SyncInfo(
                on_wait=[], on_update=[_sync_update(sid, "sem-sub-imm", eng_count[eng])]
            )
            tail.append(nop)
        last_blk.instructions = tail
    return sum(eng_count.values())


def build(t_steps=T, split_waits=True):
    nc = bass.Bass()
    spare_sems = [nc.alloc_semaphore(f"splitspare{i}") for i in range(48)]

    sent_e = nc.declare_dram_parameter("sent", [BC, T], I32, isOutput=False)
    lensm1_e = nc.declare_dram_parameter("lensm1", [BC, 1], F32, isOutput=False)
    # emb host-padded to [V, 384]: col 300 = 1.0 (bias ones-col), 301: = 0,
    # so gathered rows arrive fully formed (no per-tile memsets, and each
    # xbar transpose depends only on its own gather)
    emb_e = nc.declare_dram_parameter("emb", [V, 3 * P], BF16, isOutput=False)
    wx_e = nc.declare_dram_parameter("wx", [3 * P, G], BF16, isOutput=False)
    wh_e = nc.declare_dram_parameter("wh", [3 * P, G], BF16, isOutput=False)
    wout_e = nc.declare_dram_parameter("wout", [H, C], F32, isOutput=False)
    bout_e = nc.declare_dram_parameter("bout", [1, C], F32, isOutput=False)
    arange_e = nc.declare_dram_parameter("arange", [1, T], F32, isOutput=False)
    out_e = nc.declare_dram_parameter("out", [BC, C], F32, isOutput=True)

    with tile.TileContext(nc) as tc:
        with (
            tc.tile_pool(name="const", bufs=1) as const,
            tc.tile_pool(name="wpool", bufs=1) as wpool,
            tc.tile_pool(name="xtp", bufs=1) as xtp,
            tc.tile_pool(name="work", bufs=2) as work,
            tc.tile_pool(name="psum", bufs=2, space="PSUM") as psum,
        ):
            # ---- sent first: the gather pipeline depends on it ----
            sent_sb = const.tile([BC, T], I32)
            nc.sync.dma_start(out=sent_sb[:], in_=sent_e[:])

            ident = const.tile([P, P], F32)
            identb = const.tile([P, P], BF16)

            # x staging: one standalone tile per timestep, written only by
            # its gather and read only by its transpose -- zero cross-step
            # dependencies, so both DMA queues free-run ahead of the scan.
            xt_all = xtp.tile([P, 3, t_steps * P], BF16, name="xt_all")
            xpads = [
                xtp.tile([P, 3 * P], BF16, name=f"xpad{i}")
                for i in range(t_steps)
            ]

            def prep_gather(t):
                nc.gpsimd.indirect_dma_start(
                    out=xpads[t][:, 0 : 3 * P], out_offset=None, in_=emb_e[:],
                    in_offset=IndirectOffsetOnAxis(ap=sent_sb[:, t : t + 1], axis=0),
                )

            def prep_transpose(t):
                nc.sync.dma_start_transpose(
                    out=xt_all[:, :, t * P : (t + 1) * P], in_=xpads[t][:]
                )

            # interleave gather/transpose emission pairwise so the static
            # schedule pipelines the two queues
            for u in range(min(4, t_steps)):
                prep_gather(u)
                prep_transpose(u)
            # weights on the tensor/scalar/vector DMA queues, in parallel
            # with the gathers and transposes
            wq = [nc.scalar, nc.scalar, nc.scalar]
            wx_t = []
            for k in range(3):
                wxk = wpool.tile([P, G], BF16, name=f"wx{k}")
                wq[k].dma_start(out=wxk[:], in_=wx_e[k * P : (k + 1) * P, :])
                wx_t.append(wxk)
            wh_t = []
            for k in range(3):
                whk = wpool.tile([P, G], BF16, name=f"wh{k}")
                wq[k].dma_start(out=whk[:], in_=wh_e[k * P : (k + 1) * P, :])
                wh_t.append(whk)
            wout_t = []
            for k, (k0, k1) in enumerate(KH):
                kk = k1 - k0
                wok = wpool.tile([P, C], F32, name=f"wout{k}")
                wq[k].dma_start(out=wok[:kk, :], in_=wout_e[k0:k1, :])
                wout_t.append(wok)

            make_identity(nc, ident)
            nc.vector.tensor_copy(out=identb[:], in_=ident[:])

            lensm1 = const.tile([BC, 1], F32)
            nc.scalar.dma_start(out=lensm1[:], in_=lensm1_e[:])

            arange_sb = const.tile([BC, T], F32)
            arange_bcast = bass.AP(
                tensor=arange_e, offset=0, ap=[[0, BC], [1, T]]
            )
            nc.gpsimd.dma_start(out=arange_sb[:], in_=arange_bcast)

            # delta[b,t] = (t == lens[b]-1), as f32
            delta = const.tile([BC, T], F32)
            nc.vector.tensor_scalar(
                out=delta[:], in0=arange_sb[:], scalar1=lensm1[:, 0:1],
                scalar2=None, op0=mybir.AluOpType.is_equal,
            )

            bout_sb = const.tile([BC, C], F32)
            bout_bcast = bass.AP(
                tensor=bout_e, offset=0, ap=[[0, BC], [1, C]]
            )
            nc.gpsimd.dma_start(out=bout_sb[:], in_=bout_bcast)

            for u in range(4, t_steps):
                prep_gather(u)
                prep_transpose(u)

            # preload the sigmoid/tanh ACT table set during the prologue
            actpre = const.tile([BC, 1], BF16)
            nc.scalar.activation(
                out=actpre[:, 0:1], in_=lensm1[:, 0:1],
                func=mybir.ActivationFunctionType.Sigmoid,
            )

            # ---- scan state ----
            hout = const.tile([BC, H], F32)
            nc.vector.memset(hout[:], 0.0)

            # h_new pair: pads [H:384] pre-zeroed once, alternate per step
            h_news = []
            for i in range(2):
                hn = const.tile([BC, 3 * P], BF16, name=f"h_new{i}")
                nc.vector.memset(hn[:, D : 3 * P], 0.0)
                h_news.append(hn)

            # per-gate PSUM tiles: f/i double-buffered (x-prefill right after
            # h-matmuls), g/o single-buffered (x-prefill after their reader)
            psg = psum.tile([BC, H], F32, name="psgg", tag="psg", bufs=1)
            pso = psum.tile([BC, H], F32, name="psoo", tag="pso", bufs=1)

            def x_mms(t, ps, nr, stop):
                n0, n1 = nr
                for k in range(3):
                    nc.tensor.matmul(
                        out=ps[:, 0:H],
                        lhsT=xt_all[:, k, t * P : (t + 1) * P],
                        rhs=wx_t[k][:, n0:n1],
                        start=(k == 0),
                        stop=(stop and k == 2),
                    )

            def h_mms(ps, ht, nr):
                n0, n1 = nr
                for k in range(3):
                    nc.tensor.matmul(
                        out=ps[:, 0:H],
                        lhsT=ht[:, k, :],
                        rhs=wh_t[k][:, n0:n1],
                        start=False,
                        stop=(k == 2),
                    )

            psf_cur = psum.tile([BC, H], F32, name="psf0", tag="psf")
            psi_cur = psum.tile([BC, H], F32, name="psi0", tag="psi")
            x_mms(0, psf_cur, NF, stop=True)
            x_mms(0, psi_cur, NI, stop=True)
            x_mms(0, psg, NG, stop=True)
            x_mms(0, pso, NO, stop=True)

            from bass_rust import add_dep_helper

            c_prev = None
            ht_prev = None
            for t in range(t_steps):
                last = t + 1 >= t_steps
                if ht_prev is not None:
                    h_mms(psf_cur, ht_prev, NF)
                    h_mms(psi_cur, ht_prev, NI)
                    h_mms(psg, ht_prev, NG)
                    h_mms(pso, ht_prev, NO)

                # f first: t2 = f*c only needs f
                tf = work.tile([BC, H], BF16, name="tf", tag="tf")
                nc.scalar.activation(
                    out=tf[:], in_=psf_cur[:, 0:H],
                    func=mybir.ActivationFunctionType.Sigmoid,
                )
                psf_next = None
                psi_next = None
                if not last:
                    psf_next = psum.tile([BC, H], F32, name="psf", tag="psf")
                    x_mms(t + 1, psf_next, NF, stop=False)
                    psi_next = psum.tile([BC, H], F32, name="psi", tag="psi")
                    x_mms(t + 1, psi_next, NI, stop=False)

                ti = work.tile([BC, H], BF16, name="ti", tag="ti")
                nc.scalar.activation(
                    out=ti[:], in_=psi_cur[:, 0:H],
                    func=mybir.ActivationFunctionType.Sigmoid,
                )
                if c_prev is not None:
                    t2_ = work.tile([BC, H], BF16, name="t2_", tag="t2_")
                    nc.vector.tensor_mul(t2_[:], tf[:], c_prev[:])

                tg = work.tile([BC, H], BF16, name="tg", tag="tg")
                nc.scalar.activation(
                    out=tg[:], in_=psg[:, 0:H],
                    func=mybir.ActivationFunctionType.Tanh,
                )
                if not last:
                    x_mms(t + 1, psg, NG, stop=False)

                t1_ = work.tile([BC, H], BF16, name="t1_", tag="t1_")
                nc.vector.tensor_mul(t1_[:], ti[:], tg[:])

                to = work.tile([BC, H], BF16, name="to", tag="to")
                nc.scalar.activation(
                    out=to[:], in_=pso[:, 0:H],
                    func=mybir.ActivationFunctionType.Sigmoid,
                )
                if not last:
                    x_mms(t + 1, pso, NO, stop=False)

                if c_prev is None:
                    c_new = t1_
                else:
                    c_new = work.tile([BC, H], BF16, name="c_new", tag="c_new")
                    nc.vector.tensor_add(c_new[:], t1_[:], t2_[:])

                tc_ = work.tile([BC, H], BF16, name="tc_", tag="tc_")
                nc.scalar.activation(
                    out=tc_[:], in_=c_new[:], func=mybir.ActivationFunctionType.Tanh
                )
                h_new = h_news[t % 2]
                nc.vector.tensor_mul(h_new[:, 0:H], to[:], tc_[:])

                # transpose h into ONE PSUM tile + ONE copy: all 12 next-step
                # h-matmuls become ready at once, so the static scheduler
                # keeps the f,i,g,o emission order (f's stop lands 3rd)
                last_copy = None
                if not last:
                    ht = work.tile([P, 3, P], BF16, name="ht", tag="ht")
                    trp = psum.tile([P, 3 * P], BF16, name="trp", tag="trp",
                                    bufs=1)
                    for k in range(3):
                        nc.tensor.transpose(
                            out=trp[:, k * P : (k + 1) * P],
                            in_=h_new[:, k * P : (k + 1) * P],
                            identity=identb[:],
                        )
                    last_copy = nc.vector.tensor_copy(out=ht[:, :, :], in_=trp[:])
                    ht_prev = ht

                # hout += delta_t * h  (off the critical chain; pinned after
                # the ht copy so it can't steal the DVE slot before it)
                cap = nc.vector.scalar_tensor_tensor(
                    out=hout[:], in0=h_new[:, 0:H], scalar=delta[:, t : t + 1],
                    in1=hout[:], op0=mybir.AluOpType.mult, op1=mybir.AluOpType.add,
                )
                if last_copy is not None:
                    add_dep_helper(cap.ins, last_copy.ins, sync=False,
                                   reason="capture after ht copy")
                c_prev = c_new
                psf_cur = psf_next
                psi_cur = psi_next

            # ---- output projection (f32) ----
            hot = work.tile([P, 3, P], F32, name="hot")
            for k, (k0, k1) in enumerate(KH):
                kk = k1 - k0
                trp = psum.tile([P, P], F32, name="trpo", tag="trp", bufs=1)
                nc.tensor.transpose(
                    out=trp[:kk, :], in_=hout[:, k0:k1], identity=ident[:]
                )
                nc.vector.tensor_copy(out=hot[:kk, k, :], in_=trp[:kk, :])
            po = psum.tile([P, P], F32, name="po", tag="pso", bufs=1)
            for k, (k0, k1) in enumerate(KH):
                kk = k1 - k0
                nc.tensor.matmul(
                    out=po[:, 0:C],
                    lhsT=hot[:kk, k, :],
                    rhs=wout_t[k][:kk, :],
                    start=(k == 0),
                    stop=(k == 2),
                )
            logit = work.tile([BC, C], F32, name="logit")
            nc.vector.tensor_add(logit[:], po[:, 0:C], bout_sb[:])
            nc.sync.dma_start(out=out_e[:], in_=logit[:])

    if split_waits:
        _split_multi_waits(nc, [s.num for s in spare_sems])
    return nc


_NC_CACHE = {}


def _get_nc(t_steps=T):
    if t_steps not in _NC_CACHE:
        _NC_CACHE[t_steps] = build(t_steps)
    return _NC_CACHE[t_steps]


def make_in_maps(sent, lens, emb, Wx, Wh, b, Wout, bout):
    # permute gate columns [i|f|g|o] -> [f|i|g|o]
    perm = np.concatenate(
        [np.arange(300, 600), np.arange(0, 300), np.arange(600, 900),
         np.arange(900, 1200)]
    )
    wx_aug = np.concatenate(
        [np.asarray(Wx, np.float32)[:, perm],
         np.asarray(b, np.float32)[perm][None, :],
         np.zeros((384 - D - 1, G), np.float32)], axis=0
    ).astype(ml_dtypes.bfloat16)
    wh_p = np.concatenate(
        [np.asarray(Wh, np.float32)[:, perm],
         np.zeros((384 - H, G), np.float32)], axis=0
    ).astype(ml_dtypes.bfloat16)
    embf = np.asarray(emb, np.float32)
    emb_pad = np.zeros((V, 3 * P), np.float32)
    emb_pad[:, :D] = embf
    emb_pad[:, D] = 1.0
    emb = np.ascontiguousarray(emb_pad.astype(ml_dtypes.bfloat16))
    wout = np.ascontiguousarray(np.asarray(Wout, np.float32))
    bout2 = np.asarray(bout, np.float32).reshape(1, C)
    arange = np.arange(T, dtype=np.float32).reshape(1, T)

    in_maps = []
    for i in range(N_CORES):
        sl = slice(i * BC, (i + 1) * BC)
        in_maps.append({
            "sent": np.ascontiguousarray(np.asarray(sent, np.int32)[sl]),
            "lensm1": (np.asarray(lens, np.int32)[sl] - 1).reshape(BC, 1).astype(np.float32),
            "emb": emb,
            "wx": np.ascontiguousarray(wx_aug),
            "wh": wh_p,
            "wout": wout,
            "bout": bout2,
            "arange": arange,
        })
    return in_maps


def kernel(sent, lens, emb, Wx, Wh, b, Wout, bout):
    nc = _get_nc(T)
    in_maps = make_in_maps(sent, lens, emb, Wx, Wh, b, Wout, bout)
    res = run_bass_kernel_spmd(nc, in_maps, core_ids=list(range(N_CORES)))
    out = np.concatenate(
        [res.results[i]["out"] for i in range(N_CORES)], axis=0
    )
    return out.astype(np.float32)


# revision 9
# speedup vs baseline: 1.0514x; 1.0514x over previous
"""Trainium2 Bass kernel for masked-LSTM sentence classifier (nn_ABSA_Lstm).

Data-parallel over 8 NeuronCores, 128 sentences per core.

v3: gate column order [f | i | g | o] (host-permuted), one PSUM tile per
gate so the f-sigmoid starts after only its own 3 h-matmuls and the ACT
chain runs f -> i -> g -> o -> tanh(c) with the DVE cell update
(t2 = f*c, t1 = i*g, c = t1+t2, h = o*tanh(c)) interleaved.

Per core:
  - x path (pipelined LAG steps ahead): one indirect DMA per timestep
    gathers emb[sent[:,t]] from a bf16 embedding table (host-cast) into a
    ring of padded bf16 staging tiles; xbar DMA transposes (bursts, Sync
    queue reserved for them) produce xT with a constant ones-row so the
    gate bias rides along in the x matmul (Wx host-augmented with b,
    zero-padded to K=384).  Weight/const DMAs ride the tensor/scalar/
    vector engine queues so they don't delay the first transposes.
  - scan t = 0..T-1 (bf16 matmuls + elementwise, fp32 PSUM): x-parts for
    step t+1 prefill the f/i tiles (double-buffered) right after step t's
    h-matmuls as PE filler; g/o tiles are single-buffered and refill after
    their ACT readers.  h transposed on PE into ONE PSUM tile with ONE DVE
    copy so all 12 next-step h-matmuls become ready together (keeps the
    scheduler from k-major reordering them, which would delay sig_f).
    h captured into hout at t == len-1 via one fused per-partition
    mul-add, scheduling-pinned after the ht copy.
  - logit = hout @ Wout + bout (f32 matmul), one DMA out.

A post-pass splits multi-wait instructions (this walrus accepts one sync
wait per instruction): engine ops spill waits onto same-engine NoOps; DMA
queue ops route spilled waits through dedicated semaphores that are
restored to zero at the kernel tail (NEFF stays re-executable).
"""

import sys

for _p in ("/opt/trn_rl_repo", "/root/.axon_site/_ro/trn_rl_repo"):
    if _p not in sys.path:
        sys.path.append(_p)

import numpy as np
import ml_dtypes

from concourse import bass, mybir
import concourse.tile as tile
from concourse.bass import IndirectOffsetOnAxis
from concourse.bass_utils import run_bass_kernel_spmd
from concourse.masks import make_identity

B, T, V, D, H, C = 1024, 80, 50000, 300, 300, 3
G = 4 * H            # 1200 gate columns, order [f | i | g | o]
DA = D + 1           # 301: ones-row for bias
N_CORES = 8
BC = B // N_CORES    # 128 sentences per core
P = 128

F32 = mybir.dt.float32
BF16 = mybir.dt.bfloat16
I32 = mybir.dt.int32

KH = [(0, 128), (128, 256), (256, H)]        # h-side contraction tiles
# gate column ranges in the permuted weights
NF, NI, NG, NO = (0, H), (H, 2 * H), (2 * H, 3 * H), (3 * H, G)


def _sync_wait(sem_id, value):
    import bass_rust
    return bass_rust.SyncWait(
        sync_type="semaphore", id=sem_id, ant_name=f"splitsem_{sem_id}",
        wait_mode="sem-ge-imm", wait_value=value, wait_reg=None,
    )


def _sync_update(sem_id, mode, value):
    import bass_rust
    return bass_rust.SyncUpdate(
        sync_type="semaphore", id=sem_id, ant_name=f"splitsem_{sem_id}",
        update_mode=mode, update_value=value, update_reg=None,
    )


def _split_multi_waits(nc, spare_sem_ids):
    """walrus caps sync waits per instruction at 1 for every struct we hit.

    Engine instructions: spill excess waits onto single-wait NoOps placed
    just before, on the same engine (engine streams are in-order).

    DMA/queue instructions: a preceding engine NoOp may not order the DGE
    ring, so the spill NoOps perform ALL the original waits and the last one
    increments a dedicated semaphore; the DMA's single wait becomes that
    semaphore. Each such semaphore is decremented back to 0 at the kernel
    tail so repeated NEFF executions stay correct."""
    f = nc.m.functions[0]
    spare = list(spare_sem_ids)
    eng_sem = {}     # engine -> sem id (one per issuing engine, in-order stream)
    eng_count = {}   # engine -> number of increments so far
    for blk in f.blocks:
        out = []
        for ins in blk.instructions:
            si = ins.sync_info
            waits = list(si.on_wait) if si and si.on_wait else []
            if len(waits) <= 1:
                out.append(ins)
                continue
            tname = type(ins).__name__
            is_dma = ("DMA" in tname or "TensorLoad" in tname
                      or "TensorSave" in tname)
            if is_dma:
                eng = ins.engine
                if eng not in eng_sem:
                    eng_sem[eng] = spare.pop()
                    eng_count[eng] = 0
                sid = eng_sem[eng]
                eng_count[eng] += 1
                target = eng_count[eng]
                for j, w in enumerate(waits):
                    nop = mybir.InstNoOp(name=f"nop-dsplit-{nc.next_id()}")
                    nop.engine = eng
                    upd = [_sync_update(sid, "sem-inc", 1)] if j == len(waits) - 1 else []
                    nop.sync_info = mybir.SyncInfo(on_wait=[w], on_update=upd)
                    out.append(nop)
                si.on_wait = [_sync_wait(sid, target)]
            else:
                for w in waits[:-1]:
                    nop = mybir.InstNoOp(name=f"nop-split-{nc.next_id()}")
                    nop.engine = ins.engine
                    nop.sync_info = mybir.SyncInfo(on_wait=[w], on_update=[])
                    out.append(nop)
                si.on_wait = waits[-1:]
            out.append(ins)
        blk.instructions = out
    # tail: restore spilled-DMA semaphores to 0 for repeat executions
    if eng_sem:
        last_blk = f.blocks[-1]
        tail = list(last_blk.instructions)
        for eng, sid in eng_sem.items():
            nop = mybir.InstNoOp(name=f"nop-dclear-{nc.next_id()}")
            nop.engine = mybir.EngineType.SP
            nop.sync_info = mybir.SyncInfo(
                on_wait=[], on_update=[_sync_update(sid, "sem-sub-imm", eng_count[eng])]
            )
            tail.append(nop)
        last_blk.instructions = tail
    return sum(eng_count.values())


def build(t_steps=T, split_waits=True):
    nc = bass.Bass()
    spare_sems = [nc.alloc_semaphore(f"splitspare{i}") for i in range(48)]

    sent_e = nc.declare_dram_parameter("sent", [BC, T], I32, isOutput=False)
    lensm1_e = nc.declare_dram_parameter("lensm1", [BC, 1], F32, isOutput=False)
    # emb host-padded to [V, 384]: col 300 = 1.0 (bias ones-col), 301: = 0,
    # so gathered rows arrive fully formed (no per-tile memsets, and each
    # xbar transpose depends only on its own gather)
    emb_e = nc.declare_dram_parameter("emb", [V, 3 * P], BF16, isOutput=False)
    wx_e = nc.declare_dram_parameter("wx", [3 * P, G], BF16, isOutput=False)
    wh_e = nc.declare_dram_parameter("wh", [3 * P, G], BF16, isOutput=False)
    wout_e = nc.declare_dram_parameter("wout", [H, C], F32, isOutput=False)
    bout_e = nc.declare_dram_parameter("bout", [1, C], F32, isOutput=False)
    arange_e = nc.declare_dram_parameter("arange", [1, T], F32, isOutput=False)
    out_e = nc.declare_dram_parameter("out", [BC, C], F32, isOutput=True)

    with tile.TileContext(nc) as tc:
        with (
            tc.tile_pool(name="const", bufs=1) as const,
            tc.tile_pool(name="wpool", bufs=1) as wpool,
            tc.tile_pool(name="xtp", bufs=1) as xtp,
            tc.tile_pool(name="work", bufs=2) as work,
            tc.tile_pool(name="psum", bufs=2, space="PSUM") as psum,
        ):
            # ---- sent first: the gather pipeline depends on it ----
            sent_sb = const.tile([BC, T], I32)
            nc.sync.dma_start(out=sent_sb[:], in_=sent_e[:])

            ident = const.tile([P, P], F32)
            identb = const.tile([P, P], BF16)

            # x staging: one standalone tile per timestep, written only by
            # its gather and read only by its transpose -- zero cross-step
            # dependencies, so both DMA queues free-run ahead of the scan.
            xts = [
                xtp.tile([P, 3, P], BF16, name=f"xt{i}")
                for i in range(t_steps)
            ]
            xpads = [
                xtp.tile([P, 3 * P], BF16, name=f"xpad{i}")
                for i in range(t_steps)
            ]

            def prep_gather(t):
                nc.gpsimd.indirect_dma_start(
                    out=xpads[t][:, 0 : 3 * P], out_offset=None, in_=emb_e[:],
                    in_offset=IndirectOffsetOnAxis(ap=sent_sb[:, t : t + 1], axis=0),
                )

            def prep_transpose(t):
                nc.sync.dma_start_transpose(
                    out=xts[t][:, :, :], in_=xpads[t][:]
                )

            # interleave gather/transpose emission pairwise so the static
            # schedule pipelines the two queues
            for u in range(min(4, t_steps)):
                prep_gather(u)
                prep_transpose(u)
            # weights on the tensor/scalar/vector DMA queues, in parallel
            # with the gathers and transposes
            wq = [nc.scalar, nc.scalar, nc.scalar]
            wx_t = []
            for k in range(3):
                wxk = wpool.tile([P, G], BF16, name=f"wx{k}")
                wq[k].dma_start(out=wxk[:], in_=wx_e[k * P : (k + 1) * P, :])
                wx_t.append(wxk)
            wh_t = []
            for k in range(3):
                whk = wpool.tile([P, G], BF16, name=f"wh{k}")
                wq[k].dma_start(out=whk[:], in_=wh_e[k * P : (k + 1) * P, :])
                wh_t.append(whk)
            wout_t = []
            for k, (k0, k1) in enumerate(KH):
                kk = k1 - k0
                wok = wpool.tile([P, C], F32, name=f"wout{k}")
                wq[k].dma_start(out=wok[:kk, :], in_=wout_e[k0:k1, :])
                wout_t.append(wok)

            make_identity(nc, ident)
            nc.vector.tensor_copy(out=identb[:], in_=ident[:])

            lensm1 = const.tile([BC, 1], F32)
            nc.scalar.dma_start(out=lensm1[:], in_=lensm1_e[:])

            arange_sb = const.tile([BC, T], F32)
            arange_bcast = bass.AP(
                tensor=arange_e, offset=0, ap=[[0, BC], [1, T]]
            )
            nc.gpsimd.dma_start(out=arange_sb[:], in_=arange_bcast)

            # delta[b,t] = (t == lens[b]-1), as f32
            delta = const.tile([BC, T], F32)
            nc.vector.tensor_scalar(
                out=delta[:], in0=arange_sb[:], scalar1=lensm1[:, 0:1],
                scalar2=None, op0=mybir.AluOpType.is_equal,
            )

            bout_sb = const.tile([BC, C], F32)
            bout_bcast = bass.AP(
                tensor=bout_e, offset=0, ap=[[0, BC], [1, C]]
            )
            nc.gpsimd.dma_start(out=bout_sb[:], in_=bout_bcast)

            for u in range(4, t_steps):
                prep_gather(u)
                prep_transpose(u)

            # preload the sigmoid/tanh ACT table set during the prologue
            actpre = const.tile([BC, 1], BF16)
            nc.scalar.activation(
                out=actpre[:, 0:1], in_=lensm1[:, 0:1],
                func=mybir.ActivationFunctionType.Sigmoid,
            )

            # ---- scan state ----
            hout = const.tile([BC, H], F32)
            nc.vector.memset(hout[:], 0.0)

            # h_new pair: pads [H:384] pre-zeroed once, alternate per step
            h_news = []
            for i in range(2):
                hn = const.tile([BC, 3 * P], BF16, name=f"h_new{i}")
                nc.vector.memset(hn[:, D : 3 * P], 0.0)
                h_news.append(hn)

            # per-gate PSUM tiles: f/i double-buffered (x-prefill right after
            # h-matmuls), g/o single-buffered (x-prefill after their reader)
            psg = psum.tile([BC, H], F32, name="psgg", tag="psg", bufs=1)
            pso = psum.tile([BC, H], F32, name="psoo", tag="pso", bufs=1)

            def x_mms(t, ps, nr, stop):
                n0, n1 = nr
                for k in range(3):
                    nc.tensor.matmul(
                        out=ps[:, 0:H],
                        lhsT=xts[t][:, k, :],
                        rhs=wx_t[k][:, n0:n1],
                        start=(k == 0),
                        stop=(stop and k == 2),
                    )

            def h_mms(ps, ht, nr):
                n0, n1 = nr
                for k in range(3):
                    nc.tensor.matmul(
                        out=ps[:, 0:H],
                        lhsT=ht[:, k, :],
                        rhs=wh_t[k][:, n0:n1],
                        start=False,
                        stop=(k == 2),
                    )

            psf_cur = psum.tile([BC, H], F32, name="psf0", tag="psf")
            psi_cur = psum.tile([BC, H], F32, name="psi0", tag="psi")
            x_mms(0, psf_cur, NF, stop=True)
            x_mms(0, psi_cur, NI, stop=True)
            x_mms(0, psg, NG, stop=True)
            x_mms(0, pso, NO, stop=True)

            from bass_rust import add_dep_helper

            c_prev = None
            ht_prev = None
            for t in range(t_steps):
                last = t + 1 >= t_steps
                if ht_prev is not None:
                    h_mms(psf_cur, ht_prev, NF)
                    h_mms(psi_cur, ht_prev, NI)
                    h_mms(psg, ht_prev, NG)
                    h_mms(pso, ht_prev, NO)

                # f first: t2 = f*c only needs f
                tf = work.tile([BC, H], BF16, name="tf", tag="tf")
                nc.scalar.activation(
                    out=tf[:], in_=psf_cur[:, 0:H],
                    func=mybir.ActivationFunctionType.Sigmoid,
                )
                psf_next = None
                psi_next = None
                if not last:
                    psf_next = psum.tile([BC, H], F32, name="psf", tag="psf")
                    x_mms(t + 1, psf_next, NF, stop=False)
                    psi_next = psum.tile([BC, H], F32, name="psi", tag="psi")
                    x_mms(t + 1, psi_next, NI, stop=False)

                ti = work.tile([BC, H], BF16, name="ti", tag="ti")
                nc.scalar.activation(
                    out=ti[:], in_=psi_cur[:, 0:H],
                    func=mybir.ActivationFunctionType.Sigmoid,
                )
                if c_prev is not None:
                    t2_ = work.tile([BC, H], BF16, name="t2_", tag="t2_")
                    nc.vector.tensor_mul(t2_[:], tf[:], c_prev[:])

                tg = work.tile([BC, H], BF16, name="tg", tag="tg")
                nc.scalar.activation(
                    out=tg[:], in_=psg[:, 0:H],
                    func=mybir.ActivationFunctionType.Tanh,
                )
                if not last:
                    x_mms(t + 1, psg, NG, stop=False)

                t1_ = work.tile([BC, H], BF16, name="t1_", tag="t1_")
                nc.vector.tensor_mul(t1_[:], ti[:], tg[:])

                to = work.tile([BC, H], BF16, name="to", tag="to")
                nc.scalar.activation(
                    out=to[:], in_=pso[:, 0:H],
                    func=mybir.ActivationFunctionType.Sigmoid,
                )
                if not last:
                    x_mms(t + 1, pso, NO, stop=False)

                if c_prev is None:
                    c_new = t1_
                else:
                    c_new = work.tile([BC, H], BF16, name="c_new", tag="c_new")
                    nc.vector.tensor_add(c_new[:], t1_[:], t2_[:])

                tc_ = work.tile([BC, H], BF16, name="tc_", tag="tc_")
                nc.scalar.activation(
                    out=tc_[:], in_=c_new[:], func=mybir.ActivationFunctionType.Tanh
                )
                h_new = h_news[t % 2]
                nc.vector.tensor_mul(h_new[:, 0:H], to[:], tc_[:])

                # transpose h into ONE PSUM tile + ONE copy: all 12 next-step
                # h-matmuls become ready at once, so the static scheduler
                # keeps the f,i,g,o emission order (f's stop lands 3rd)
                last_copy = None
                if not last:
                    ht = work.tile([P, 3, P], BF16, name="ht", tag="ht")
                    trp = psum.tile([P, 3 * P], BF16, name="trp", tag="trp",
                                    bufs=1)
                    for k in range(3):
                        nc.tensor.transpose(
                            out=trp[:, k * P : (k + 1) * P],
                            in_=h_new[:, k * P : (k + 1) * P],
                            identity=identb[:],
                        )
                    last_copy = nc.vector.tensor_copy(out=ht[:, :, :], in_=trp[:])
                    ht_prev = ht

                # hout += delta_t * h  (off the critical chain; pinned after
                # the ht copy so it can't steal the DVE slot before it)
                cap = nc.vector.scalar_tensor_tensor(
                    out=hout[:], in0=h_new[:, 0:H], scalar=delta[:, t : t + 1],
                    in1=hout[:], op0=mybir.AluOpType.mult, op1=mybir.AluOpType.add,
                )
                if last_copy is not None:
                    add_dep_helper(cap.ins, last_copy.ins, sync=False,
                                   reason="capture after ht copy")
                c_prev = c_new
                psf_cur = psf_next
                psi_cur = psi_next

            # ---- output projection (f32) ----
            hot = work.tile([P, 3, P], F32, name="hot")
            for k, (k0, k1) in enumerate(KH):
                kk = k1 - k0
                trp = psum.tile([P, P], F32, name="trpo", tag="trp", bufs=1)
                nc.tensor.transpose(
                    out=trp[:kk, :], in_=hout[:, k0:k1], identity=ident[:]
                )
                nc.vector.tensor_copy(out=hot[:kk, k, :], in_=trp[:kk, :])
            po = psum.tile([P, P], F32, name="po", tag="pso", bufs=1)
            for k, (k0, k1) in enumerate(KH):
                kk = k1 - k0
                nc.tensor.matmul(
                    out=po[:, 0:C],
                    lhsT=hot[:kk, k, :],
                    rhs=wout_t[k][:kk, :],
                    start=(k == 0),
                    stop=(k == 2),
                )
            logit = work.tile([BC, C], F32, name="logit")
            nc.vector.tensor_add(logit[:], po[:, 0:C], bout_sb[:])
            nc.sync.dma_start(out=out_e[:], in_=logit[:])

    if split_waits:
        _split_multi_waits(nc, [s.num for s in spare_sems])
    return nc


_NC_CACHE = {}


def _get_nc(t_steps=T):
    if t_steps not in _NC_CACHE:
        _NC_CACHE[t_steps] = build(t_steps)
    return _NC_CACHE[t_steps]


def make_in_maps(sent, lens, emb, Wx, Wh, b, Wout, bout):
    # permute gate columns [i|f|g|o] -> [f|i|g|o]
    perm = np.concatenate(
        [np.arange(300, 600), np.arange(0, 300), np.arange(600, 900),
         np.arange(900, 1200)]
    )
    wx_aug = np.concatenate(
        [np.asarray(Wx, np.float32)[:, perm],
         np.asarray(b, np.float32)[perm][None, :],
         np.zeros((384 - D - 1, G), np.float32)], axis=0
    ).astype(ml_dtypes.bfloat16)
    wh_p = np.concatenate(
        [np.asarray(Wh, np.float32)[:, perm],
         np.zeros((384 - H, G), np.float32)], axis=0
    ).astype(ml_dtypes.bfloat16)
    embf = np.asarray(emb, np.float32)
    emb_pad = np.zeros((V, 3 * P), np.float32)
    emb_pad[:, :D] = embf
    emb_pad[:, D] = 1.0
    emb = np.ascontiguousarray(emb_pad.astype(ml_dtypes.bfloat16))
    wout = np.ascontiguousarray(np.asarray(Wout, np.float32))
    bout2 = np.asarray(bout, np.float32).reshape(1, C)
    arange = np.arange(T, dtype=np.float32).reshape(1, T)

    in_maps = []
    for i in range(N_CORES):
        sl = slice(i * BC, (i + 1) * BC)
        in_maps.append({
            "sent": np.ascontiguousarray(np.asarray(sent, np.int32)[sl]),
            "lensm1": (np.asarray(lens, np.int32)[sl] - 1).reshape(BC, 1).astype(np.float32),
            "emb": emb,
            "wx": np.ascontiguousarray(wx_aug),
            "wh": wh_p,
            "wout": wout,
            "bout": bout2,
            "arange": arange,
        })
    return in_maps


def kernel(sent, lens, emb, Wx, Wh, b, Wout, bout):
    nc = _get_nc(T)
    in_maps = make_in_maps(sent, lens, emb, Wx, Wh, b, Wout, bout)
    res = run_bass_kernel_spmd(nc, in_maps, core_ids=list(range(N_CORES)))
    out = np.concatenate(
        [res.results[i]["out"] for i in range(N_CORES)], axis=0
    )
    return out.astype(np.float32)


# revision 10
# speedup vs baseline: 1.3913x; 1.3233x over previous
"""Trainium2 Bass kernel for masked-LSTM sentence classifier (nn_ABSA_Lstm).

Data-parallel over 8 NeuronCores, 128 sentences per core.

v3: gate column order [f | i | g | o] (host-permuted), one PSUM tile per
gate so the f-sigmoid starts after only its own 3 h-matmuls and the ACT
chain runs f -> i -> g -> o -> tanh(c) with the DVE cell update
(t2 = f*c, t1 = i*g, c = t1+t2, h = o*tanh(c)) interleaved.

Per core:
  - x path (pipelined LAG steps ahead): one indirect DMA per timestep
    gathers emb[sent[:,t]] from a bf16 embedding table (host-cast) into a
    ring of padded bf16 staging tiles; xbar DMA transposes (bursts, Sync
    queue reserved for them) produce xT with a constant ones-row so the
    gate bias rides along in the x matmul (Wx host-augmented with b,
    zero-padded to K=384).  Weight/const DMAs ride the tensor/scalar/
    vector engine queues so they don't delay the first transposes.
  - scan t = 0..T-1 (bf16 matmuls + elementwise, fp32 PSUM): x-parts for
    step t+1 prefill the f/i tiles (double-buffered) right after step t's
    h-matmuls as PE filler; g/o tiles are single-buffered and refill after
    their ACT readers.  h transposed on PE into ONE PSUM tile with ONE DVE
    copy so all 12 next-step h-matmuls become ready together (keeps the
    scheduler from k-major reordering them, which would delay sig_f).
    h captured into hout at t == len-1 via one fused per-partition
    mul-add, scheduling-pinned after the ht copy.
  - logit = hout @ Wout + bout (f32 matmul), one DMA out.

A post-pass splits multi-wait instructions (this walrus accepts one sync
wait per instruction): engine ops spill waits onto same-engine NoOps; DMA
queue ops route spilled waits through dedicated semaphores that are
restored to zero at the kernel tail (NEFF stays re-executable).
"""

import sys

for _p in ("/opt/trn_rl_repo", "/root/.axon_site/_ro/trn_rl_repo"):
    if _p not in sys.path:
        sys.path.append(_p)

import numpy as np
import ml_dtypes

from concourse import bass, mybir
import concourse.tile as tile
from concourse.bass import IndirectOffsetOnAxis
from concourse.bass_utils import run_bass_kernel_spmd
from concourse.masks import make_identity

B, T, V, D, H, C = 1024, 80, 50000, 300, 300, 3
G = 4 * H            # 1200 gate columns, order [f | i | g | o]
DA = D + 1           # 301: ones-row for bias
N_CORES = 8
BC = B // N_CORES    # 128 sentences per core
P = 128

F32 = mybir.dt.float32
BF16 = mybir.dt.bfloat16
I32 = mybir.dt.int32

KH = [(0, 128), (128, 256), (256, H)]        # h-side contraction tiles
# gate column ranges in the permuted weights
NF, NI, NG, NO = (0, H), (H, 2 * H), (2 * H, 3 * H), (3 * H, G)


def _sync_wait(sem_id, value):
    import bass_rust
    return bass_rust.SyncWait(
        sync_type="semaphore", id=sem_id, ant_name=f"splitsem_{sem_id}",
        wait_mode="sem-ge-imm", wait_value=value, wait_reg=None,
    )


def _sync_update(sem_id, mode, value):
    import bass_rust
    return bass_rust.SyncUpdate(
        sync_type="semaphore", id=sem_id, ant_name=f"splitsem_{sem_id}",
        update_mode=mode, update_value=value, update_reg=None,
    )


def _split_multi_waits(nc, spare_sem_ids):
    """walrus caps sync waits per instruction at 1 for every struct we hit.

    Engine instructions: spill excess waits onto single-wait NoOps placed
    just before, on the same engine (engine streams are in-order).

    DMA/queue instructions: a preceding engine NoOp may not order the DGE
    ring, so the spill NoOps perform ALL the original waits and the last one
    increments a dedicated semaphore; the DMA's single wait becomes that
    semaphore. Each such semaphore is decremented back to 0 at the kernel
    tail so repeated NEFF executions stay correct."""
    f = nc.m.functions[0]
    spare = list(spare_sem_ids)
    eng_sem = {}     # engine -> sem id (one per issuing engine, in-order stream)
    eng_count = {}   # engine -> number of increments so far
    for blk in f.blocks:
        out = []
        for ins in blk.instructions:
            si = ins.sync_info
            waits = list(si.on_wait) if si and si.on_wait else []
            if len(waits) <= 1:
                out.append(ins)
                continue
            tname = type(ins).__name__
            is_dma = ("DMA" in tname or "TensorLoad" in tname
                      or "TensorSave" in tname)
            if is_dma:
                eng = ins.engine
                if eng not in eng_sem:
                    eng_sem[eng] = spare.pop()
                    eng_count[eng] = 0
                sid = eng_sem[eng]
                eng_count[eng] += 1
                target = eng_count[eng]
                for j, w in enumerate(waits):
                    nop = mybir.InstNoOp(name=f"nop-dsplit-{nc.next_id()}")
                    nop.engine = eng
                    upd = [_sync_update(sid, "sem-inc", 1)] if j == len(waits) - 1 else []
                    nop.sync_info = mybir.SyncInfo(on_wait=[w], on_update=upd)
                    out.append(nop)
                si.on_wait = [_sync_wait(sid, target)]
            else:
                for w in waits[:-1]:
                    nop = mybir.InstNoOp(name=f"nop-split-{nc.next_id()}")
                    nop.engine = ins.engine
                    nop.sync_info = mybir.SyncInfo(on_wait=[w], on_update=[])
                    out.append(nop)
                si.on_wait = waits[-1:]
            out.append(ins)
        blk.instructions = out
    # tail: restore spilled-DMA semaphores to 0 for repeat executions
    if eng_sem:
        last_blk = f.blocks[-1]
        tail = list(last_blk.instructions)
        for eng, sid in eng_sem.items():
            nop = mybir.InstNoOp(name=f"nop-dclear-{nc.next_id()}")
            nop.engine = mybir.EngineType.SP
            nop.sync_info = mybir.SyncInfo(
                on_wait=[], on_update=[_sync_update(sid, "sem-sub-imm", eng_count[eng])]
            )
            tail.append(nop)
        last_blk.instructions = tail
    return sum(eng_count.values())


def build(t_steps=T, split_waits=True):
    nc = bass.Bass()
    spare_sems = [nc.alloc_semaphore(f"splitspare{i}") for i in range(48)]

    sent_e = nc.declare_dram_parameter("sent", [BC, T], I32, isOutput=False)
    lensm1_e = nc.declare_dram_parameter("lensm1", [BC, 1], F32, isOutput=False)
    # emb host-padded to [V, 384]: col 300 = 1.0 (bias ones-col), 301: = 0,
    # so gathered rows arrive fully formed (no per-tile memsets, and each
    # xbar transpose depends only on its own gather)
    emb_e = nc.declare_dram_parameter("emb", [V, 3 * P], BF16, isOutput=False)
    wx_e = nc.declare_dram_parameter("wx", [3 * P, G], BF16, isOutput=False)
    wh_e = nc.declare_dram_parameter("wh", [3 * P, G], BF16, isOutput=False)
    wout_e = nc.declare_dram_parameter("wout", [H, C], F32, isOutput=False)
    bout_e = nc.declare_dram_parameter("bout", [1, C], F32, isOutput=False)
    arange_e = nc.declare_dram_parameter("arange", [1, T], F32, isOutput=False)
    out_e = nc.declare_dram_parameter("out", [BC, C], F32, isOutput=True)

    with tile.TileContext(nc) as tc:
        with (
            tc.tile_pool(name="const", bufs=1) as const,
            tc.tile_pool(name="wpool", bufs=1) as wpool,
            tc.tile_pool(name="xtp", bufs=1) as xtp,
            tc.tile_pool(name="work", bufs=2) as work,
            tc.tile_pool(name="psum", bufs=2, space="PSUM") as psum,
        ):
            # ---- sent first: the gather pipeline depends on it ----
            sent_sb = const.tile([BC, T], I32)
            nc.sync.dma_start(out=sent_sb[:], in_=sent_e[:])

            ident = const.tile([P, P], F32)
            identb = const.tile([P, P], BF16)

            # x staging: one standalone tile per timestep, written only by
            # its gather and read only by its transpose -- zero cross-step
            # dependencies, so both DMA queues free-run ahead of the scan.
            # 4 timesteps per staging tile / per xbar transpose: DMA-queue
            # instructions are flow-controlled by small fixed semaphore pools
            # with ~2-3us latency per completion hop, so fewer+bigger DMAs
            # keep the supply pipeline far ahead of the scan.
            QT = 4
            nq = (t_steps + QT - 1) // QT
            xts = [
                xtp.tile([P, 3 * QT, P], BF16, name=f"xt{i}")
                for i in range(nq)
            ]
            xpads = [
                xtp.tile([P, QT * 3 * P], BF16, name=f"xpad{i}")
                for i in range(nq)
            ]

            def xt_slice(t, k):
                return xts[t // QT][:, 3 * (t % QT) + k, :]

            def prep_gather(t):
                nc.gpsimd.indirect_dma_start(
                    out=xpads[t // QT][:, (t % QT) * 3 * P : (t % QT + 1) * 3 * P],
                    out_offset=None, in_=emb_e[:],
                    in_offset=IndirectOffsetOnAxis(ap=sent_sb[:, t : t + 1], axis=0),
                )

            def prep_transpose(q):
                nc.sync.dma_start_transpose(
                    out=xts[q][:, :, :], in_=xpads[q][:]
                )

            # interleave gather/transpose emission so the static schedule
            # pipelines the two queues
            for u in range(min(QT, t_steps)):
                prep_gather(u)
            prep_transpose(0)
            # weights on the scalar DMA queue (one DMA per tensor), in
            # parallel with the gathers and transposes
            wx_sb = wpool.tile([P, 3, G], BF16, name="wx_sb")
            wx_src = bass.AP(tensor=wx_e, offset=0,
                             ap=[[G, P], [P * G, 3], [1, G]])
            nc.scalar.dma_start(out=wx_sb[:], in_=wx_src)
            wx_t = [wx_sb[:, k, :] for k in range(3)]
            wh_sb = wpool.tile([P, 3, G], BF16, name="wh_sb")
            wh_src = bass.AP(tensor=wh_e, offset=0,
                             ap=[[G, P], [P * G, 3], [1, G]])
            nc.scalar.dma_start(out=wh_sb[:], in_=wh_src)
            wh_t = [wh_sb[:, k, :] for k in range(3)]
            wout_t = []
            for k, (k0, k1) in enumerate(KH):
                kk = k1 - k0
                wok = wpool.tile([P, C], F32, name=f"wout{k}")
                nc.scalar.dma_start(out=wok[:kk, :], in_=wout_e[k0:k1, :])
                wout_t.append(wok)

            make_identity(nc, ident)
            nc.vector.tensor_copy(out=identb[:], in_=ident[:])

            lensm1 = const.tile([BC, 1], F32)
            nc.scalar.dma_start(out=lensm1[:], in_=lensm1_e[:])

            arange_sb = const.tile([BC, T], F32)
            arange_bcast = bass.AP(
                tensor=arange_e, offset=0, ap=[[0, BC], [1, T]]
            )
            nc.gpsimd.dma_start(out=arange_sb[:], in_=arange_bcast)

            # delta[b,t] = (t == lens[b]-1), as f32
            delta = const.tile([BC, T], F32)
            nc.vector.tensor_scalar(
                out=delta[:], in0=arange_sb[:], scalar1=lensm1[:, 0:1],
                scalar2=None, op0=mybir.AluOpType.is_equal,
            )

            bout_sb = const.tile([BC, C], F32)
            bout_bcast = bass.AP(
                tensor=bout_e, offset=0, ap=[[0, BC], [1, C]]
            )
            nc.gpsimd.dma_start(out=bout_sb[:], in_=bout_bcast)

            for u in range(QT, t_steps):
                prep_gather(u)
                if u % QT == QT - 1:
                    prep_transpose(u // QT)

            # preload the sigmoid/tanh ACT table set during the prologue
            actpre = const.tile([BC, 1], BF16)
            nc.scalar.activation(
                out=actpre[:, 0:1], in_=lensm1[:, 0:1],
                func=mybir.ActivationFunctionType.Sigmoid,
            )

            # ---- scan state ----
            hout = const.tile([BC, H], F32)
            nc.vector.memset(hout[:], 0.0)

            # h_new pair: pads [H:384] pre-zeroed once, alternate per step
            h_news = []
            for i in range(2):
                hn = const.tile([BC, 3 * P], BF16, name=f"h_new{i}")
                nc.vector.memset(hn[:, D : 3 * P], 0.0)
                h_news.append(hn)

            # per-gate PSUM tiles: f/i double-buffered (x-prefill right after
            # h-matmuls), g/o single-buffered (x-prefill after their reader)
            psg = psum.tile([BC, H], F32, name="psgg", tag="psg", bufs=1)
            pso = psum.tile([BC, H], F32, name="psoo", tag="pso", bufs=1)

            def x_mms(t, ps, nr, stop):
                n0, n1 = nr
                for k in range(3):
                    nc.tensor.matmul(
                        out=ps[:, 0:H],
                        lhsT=xt_slice(t, k),
                        rhs=wx_t[k][:, n0:n1],
                        start=(k == 0),
                        stop=(stop and k == 2),
                    )

            def h_mms(ps, ht, nr):
                n0, n1 = nr
                for k in range(3):
                    nc.tensor.matmul(
                        out=ps[:, 0:H],
                        lhsT=ht[:, k, :],
                        rhs=wh_t[k][:, n0:n1],
                        start=False,
                        stop=(k == 2),
                    )

            psf_cur = psum.tile([BC, H], F32, name="psf0", tag="psf")
            psi_cur = psum.tile([BC, H], F32, name="psi0", tag="psi")
            x_mms(0, psf_cur, NF, stop=True)
            x_mms(0, psi_cur, NI, stop=True)
            x_mms(0, psg, NG, stop=True)
            x_mms(0, pso, NO, stop=True)

            from bass_rust import add_dep_helper

            c_prev = None
            ht_prev = None
            for t in range(t_steps):
                last = t + 1 >= t_steps
                if ht_prev is not None:
                    h_mms(psf_cur, ht_prev, NF)
                    h_mms(psi_cur, ht_prev, NI)
                    h_mms(psg, ht_prev, NG)
                    h_mms(pso, ht_prev, NO)

                # f first: t2 = f*c only needs f
                tf = work.tile([BC, H], BF16, name="tf", tag="tf")
                nc.scalar.activation(
                    out=tf[:], in_=psf_cur[:, 0:H],
                    func=mybir.ActivationFunctionType.Sigmoid,
                )
                psf_next = None
                psi_next = None
                if not last:
                    psf_next = psum.tile([BC, H], F32, name="psf", tag="psf")
                    x_mms(t + 1, psf_next, NF, stop=False)
                    psi_next = psum.tile([BC, H], F32, name="psi", tag="psi")
                    x_mms(t + 1, psi_next, NI, stop=False)

                ti = work.tile([BC, H], BF16, name="ti", tag="ti")
                nc.scalar.activation(
                    out=ti[:], in_=psi_cur[:, 0:H],
                    func=mybir.ActivationFunctionType.Sigmoid,
                )
                if c_prev is not None:
                    t2_ = work.tile([BC, H], BF16, name="t2_", tag="t2_")
                    nc.vector.tensor_mul(t2_[:], tf[:], c_prev[:])

                tg = work.tile([BC, H], BF16, name="tg", tag="tg")
                nc.scalar.activation(
                    out=tg[:], in_=psg[:, 0:H],
                    func=mybir.ActivationFunctionType.Tanh,
                )
                if not last:
                    x_mms(t + 1, psg, NG, stop=False)

                t1_ = work.tile([BC, H], BF16, name="t1_", tag="t1_")
                nc.vector.tensor_mul(t1_[:], ti[:], tg[:])

                to = work.tile([BC, H], BF16, name="to", tag="to")
                nc.scalar.activation(
                    out=to[:], in_=pso[:, 0:H],
                    func=mybir.ActivationFunctionType.Sigmoid,
                )
                if not last:
                    x_mms(t + 1, pso, NO, stop=False)

                if c_prev is None:
                    c_new = t1_
                else:
                    c_new = work.tile([BC, H], BF16, name="c_new", tag="c_new")
                    nc.vector.tensor_add(c_new[:], t1_[:], t2_[:])

                tc_ = work.tile([BC, H], BF16, name="tc_", tag="tc_")
                nc.scalar.activation(
                    out=tc_[:], in_=c_new[:], func=mybir.ActivationFunctionType.Tanh
                )
                h_new = h_news[t % 2]
                nc.vector.tensor_mul(h_new[:, 0:H], to[:], tc_[:])

                # transpose h into ONE PSUM tile + ONE copy: all 12 next-step
                # h-matmuls become ready at once, so the static scheduler
                # keeps the f,i,g,o emission order (f's stop lands 3rd)
                last_copy = None
                if not last:
                    ht = work.tile([P, 3, P], BF16, name="ht", tag="ht")
                    trp = psum.tile([P, 3 * P], BF16, name="trp", tag="trp",
                                    bufs=1)
                    for k in range(3):
                        nc.tensor.transpose(
                            out=trp[:, k * P : (k + 1) * P],
                            in_=h_new[:, k * P : (k + 1) * P],
                            identity=identb[:],
                        )
                    last_copy = nc.vector.tensor_copy(out=ht[:, :, :], in_=trp[:])
                    ht_prev = ht

                # hout += delta_t * h  (off the critical chain; pinned after
                # the ht copy so it can't steal the DVE slot before it)
                cap = nc.vector.scalar_tensor_tensor(
                    out=hout[:], in0=h_new[:, 0:H], scalar=delta[:, t : t + 1],
                    in1=hout[:], op0=mybir.AluOpType.mult, op1=mybir.AluOpType.add,
                )
                if last_copy is not None:
                    add_dep_helper(cap.ins, last_copy.ins, sync=False,
                                   reason="capture after ht copy")
                c_prev = c_new
                psf_cur = psf_next
                psi_cur = psi_next

            # ---- output projection (f32) ----
            hot = work.tile([P, 3, P], F32, name="hot")
            for k, (k0, k1) in enumerate(KH):
                kk = k1 - k0
                trp = psum.tile([P, P], F32, name="trpo", tag="trp", bufs=1)
                nc.tensor.transpose(
                    out=trp[:kk, :], in_=hout[:, k0:k1], identity=ident[:]
                )
                nc.vector.tensor_copy(out=hot[:kk, k, :], in_=trp[:kk, :])
            po = psum.tile([P, P], F32, name="po", tag="pso", bufs=1)
            for k, (k0, k1) in enumerate(KH):
                kk = k1 - k0
                nc.tensor.matmul(
                    out=po[:, 0:C],
                    lhsT=hot[:kk, k, :],
                    rhs=wout_t[k][:kk, :],
                    start=(k == 0),
                    stop=(k == 2),
                )
            logit = work.tile([BC, C], F32, name="logit")
            nc.vector.tensor_add(logit[:], po[:, 0:C], bout_sb[:])
            nc.sync.dma_start(out=out_e[:], in_=logit[:])

    if split_waits:
        _split_multi_waits(nc, [s.num for s in spare_sems])
    return nc


_NC_CACHE = {}


def _get_nc(t_steps=T):
    if t_steps not in _NC_CACHE:
        _NC_CACHE[t_steps] = build(t_steps)
    return _NC_CACHE[t_steps]


def make_in_maps(sent, lens, emb, Wx, Wh, b, Wout, bout):
    # permute gate columns [i|f|g|o] -> [f|i|g|o]
    perm = np.concatenate(
        [np.arange(300, 600), np.arange(0, 300), np.arange(600, 900),
         np.arange(900, 1200)]
    )
    wx_aug = np.concatenate(
        [np.asarray(Wx, np.float32)[:, perm],
         np.asarray(b, np.float32)[perm][None, :],
         np.zeros((384 - D - 1, G), np.float32)], axis=0
    ).astype(ml_dtypes.bfloat16)
    wh_p = np.concatenate(
        [np.asarray(Wh, np.float32)[:, perm],
         np.zeros((384 - H, G), np.float32)], axis=0
    ).astype(ml_dtypes.bfloat16)
    embf = np.asarray(emb, np.float32)
    emb_pad = np.zeros((V, 3 * P), np.float32)
    emb_pad[:, :D] = embf
    emb_pad[:, D] = 1.0
    emb = np.ascontiguousarray(emb_pad.astype(ml_dtypes.bfloat16))
    wout = np.ascontiguousarray(np.asarray(Wout, np.float32))
    bout2 = np.asarray(bout, np.float32).reshape(1, C)
    arange = np.arange(T, dtype=np.float32).reshape(1, T)

    in_maps = []
    for i in range(N_CORES):
        sl = slice(i * BC, (i + 1) * BC)
        in_maps.append({
            "sent": np.ascontiguousarray(np.asarray(sent, np.int32)[sl]),
            "lensm1": (np.asarray(lens, np.int32)[sl] - 1).reshape(BC, 1).astype(np.float32),
            "emb": emb,
            "wx": np.ascontiguousarray(wx_aug),
            "wh": wh_p,
            "wout": wout,
            "bout": bout2,
            "arange": arange,
        })
    return in_maps


def kernel(sent, lens, emb, Wx, Wh, b, Wout, bout):
    nc = _get_nc(T)
    in_maps = make_in_maps(sent, lens, emb, Wx, Wh, b, Wout, bout)
    res = run_bass_kernel_spmd(nc, in_maps, core_ids=list(range(N_CORES)))
    out = np.concatenate(
        [res.results[i]["out"] for i in range(N_CORES)], axis=0
    )
    return out.astype(np.float32)
